# revision 28
# baseline (speedup 1.0000x reference)
"""GNS layer (edge MLP -> segment_sum -> node MLP + layernorms) on 8 trn2 cores.

Sharding: edges partitioned by *receiver* node range; core k owns nodes
[6250k, 6250(k+1)) and the edges whose receiver lands there. Local nodes are
permuted into 49 blocks of 128 via a capacity-constrained degree-balanced
packing so nearly every block needs exactly ceil(E_core/(49*128)) edge tiles;
the segment-sum is core-local (PSUM one-hot scatter) and the host un-permutes
the output rows.

Edge pipeline runs "transposed" ([feat, edge] layouts) so the big MLP matmuls
keep weights stationary and stream 512-edge tiles. Sender features are
pre-gathered on the host into a [128, S] fp8 stream (x[senders].T in edge
order) -- no on-device gather. Receiver contribution uses the zb trick:
zb = x_block @ eW1[D:2D] computed on-device per block, scattered to edges via
a one-hot matmul. One-hots are host-built, streamed in fp8 (exact for 0/1).
All matmuls run plain bf16-rate (fp8 DoubleRow tripped the chip power
throttle and was a net loss).

Host prep (cheap, numpy): sort edges by receiver, balanced node permutation,
pad each (core, block) to the shared tile profile so all 8 cores run one
SPMD program; pre-gather/transpose sender features and edge_attr; fold the
edge-layernorm affine (eg, ebeta) exactly into the node-MLP weights.
"""

import sys

sys.path.insert(0, "/opt/trn_rl_repo")

import numpy as np
import ml_dtypes

import concourse.bacc as bacc
import concourse.bass as bass
import concourse.tile as tile
from concourse import mybir
from concourse.bass_utils import run_bass_kernel_spmd

BF16 = ml_dtypes.bfloat16
FP8 = ml_dtypes.float8_e4m3

N, E, D, A, H = 50000, 500000, 128, 64, 128
NC = 8
NLOC = N // NC            # 6250 nodes per core
NB = 49                   # blocks of 128 local nodes (49*128 = 6272 >= 6250)
NPAD = NB * 128           # 6272
EPS = 1e-5
GRP = 4                   # edge tiles batched per pointwise group


def _pack_blocks(deg, T):
    """Assign NPAD nodes (with edge counts deg) to NB blocks of exactly 128
    nodes with per-block edge capacity T[j]*128. Returns [NB, 128] local node
    ids, or None if the capacity is infeasible for this profile."""
    C = T * 128
    order = np.argsort(-deg, kind="stable")
    sums = np.zeros(NB, np.int64)
    cnts = np.zeros(NB, np.int64)
    members = [[] for _ in range(NB)]
    for n in order:
        cand = np.flatnonzero(cnts < 128)
        j = cand[np.argmax(C[cand] - sums[cand])]
        members[j].append(n)
        sums[j] += deg[n]
        cnts[j] += 1
    if (sums > C).any():
        return None
    return np.array(members, dtype=np.int64)


def _host_prep(x, edge_index, edge_attr, eW1, eb1, eW2, eb2,
               nW1, nb1, nW2, nb2, eg, ebeta, ng, nbeta):
    x = np.asarray(x, np.float32)
    s = np.asarray(edge_index[0], np.int64).astype(np.int32)
    r = np.asarray(edge_index[1], np.int64).astype(np.int32)
    ea = np.asarray(edge_attr, np.float32)

    perm = np.argsort(r, kind="stable")
    rs, ss = r[perm], s[perm]
    eas = ea[perm]

    xbf = x.astype(BF16)

    # ---- per-core degree info
    degs_k, ptrs_k = [], []
    for k in range(NC):
        base = k * NLOC
        ptr = np.searchsorted(rs, np.arange(base, base + NLOC + 1))
        d = ptr[1:] - ptr[:-1]
        degs_k.append(np.concatenate([d, np.zeros(NPAD - NLOC, np.int64)]))
        ptrs_k.append(ptr)

    # shared tile profile: n11 blocks of 11 tiles, rest 10 (auto-bumped)
    tiles_needed = max(max(NB, int(np.ceil(dd.sum() / 128))) for dd in degs_k)
    base_t = max(1, tiles_needed // NB)
    n_hi = tiles_needed - base_t * NB
    for bump in range(NB + 1):
        nh = min(NB, n_hi + bump)
        T = np.array([base_t + 1] * nh + [base_t] * (NB - nh), np.int64)
        blocks = [_pack_blocks(degs_k[k], T) for k in range(NC)]
        if all(b is not None for b in blocks):
            break
    assert all(b is not None for b in blocks)

    vt = np.concatenate([[0], np.cumsum(T)])
    TT = int(vt[-1])
    S = TT * 128

    eW1 = np.asarray(eW1, np.float32)
    nW1 = np.asarray(nW1, np.float32)
    wec = np.concatenate([eW1[2 * D:], np.asarray(eb1, np.float32)[None, :]], 0)
    nW1b_eff = np.asarray(eg, np.float32)[:, None] * nW1[D:]
    c_row = np.asarray(ebeta, np.float32) @ nW1[D:]
    nbc = np.stack([np.asarray(nb1, np.float32), c_row], 0)  # [2, H]

    idn = np.eye(128, dtype=np.float32)

    common = dict(
        wea=eW1[:D].astype(BF16), web=eW1[D:2 * D].astype(BF16),
        wec=wec.astype(BF16), we2=np.asarray(eW2, np.float32).astype(BF16),
        eb2c=np.asarray(eb2, np.float32)[:, None].copy(),   # [128, 1] f32
        wna=nW1[:D].astype(BF16), wnb=nW1b_eff.astype(BF16),
        nbc=nbc.astype(BF16),
        wn2=np.asarray(nW2, np.float32).astype(BF16),
        nb2r=np.asarray(nb2, np.float32)[None, :].astype(BF16),  # [1, D]
        ngm=np.broadcast_to(np.asarray(ng, np.float32), (128, 128)).copy(),
        nbm=np.broadcast_to(np.asarray(nbeta, np.float32), (128, 128)).copy(),
        idn=idn.astype(BF16),
        onesw=np.ones((1, GRP * 128), BF16),
        epsc=np.full((128, 1), EPS, np.float32),
    )
    in_maps = []
    xposed = np.ascontiguousarray(x.T)               # [128, N] f32
    porders = []
    for k in range(NC):
        base = k * NLOC
        ptr = ptrs_k[k]
        d = ptr[1:] - ptr[:-1]
        bl = blocks[k]                               # [NB, 128] local ids

        porder = bl.reshape(-1)                      # slot -> local node id
        porders.append(porder)
        gids = np.minimum(porder + base, N - 1)
        deg_full = degs_k[k]
        degs = np.zeros((2, NPAD), np.float32)
        degs[0, :] = 1.0
        degs[1, :] = deg_full[porder]
        xtl = xposed[:, gids].astype(BF16)           # [128, NPAD]

        # edge slots, per block in slot order
        tot_e = int(d.sum())
        cols = np.empty(tot_e, np.int64)
        rloc = np.empty(tot_e, np.int64)
        eidx = np.empty(tot_e, np.int64)
        pos = 0
        for j in range(NB):
            nl = bl[j]
            real = nl < NLOC
            nlr = nl[real]
            lens = d[nlr]
            tot = int(lens.sum())
            if tot:
                starts = ptr[:-1][nlr]
                off = np.repeat(
                    starts - np.concatenate([[0], np.cumsum(lens)[:-1]]), lens)
                e = off + np.arange(tot)
                slots = np.repeat(np.flatnonzero(real), lens)
                cols[pos:pos + tot] = vt[j] * 128 + np.arange(tot)
                rloc[pos:pos + tot] = slots
                eidx[pos:pos + tot] = e
                pos += tot
        cols = cols[:pos]; rloc = rloc[:pos]; eidx = eidx[:pos]

        xsT = np.zeros((128, S), FP8)
        xsT[:, cols] = xbf[ss[eidx]].astype(FP8).T
        eaT = np.zeros((65, S), FP8)
        eaT[64, :] = 1.0
        eaT[:64, cols] = eas[eidx].astype(FP8).T
        ohd = np.zeros((128, TT, 128), FP8)
        ohd[cols % 128, cols // 128, rloc] = 1.0
        ohtd = np.zeros((128, S), FP8)
        ohtd[rloc, cols] = 1.0

        m = dict(common)
        m["xtl"] = xtl
        m["xsT"] = xsT
        m["eaT"] = eaT
        m["ohd"] = ohd
        m["ohtd"] = ohtd
        m["degs"] = degs.astype(BF16)
        in_maps.append(m)
    meta = dict(T=[int(v) for v in T], vt=[int(v) for v in vt], TT=TT,
                porders=porders)
    return in_maps, meta


def build_program(nc, meta):
    dt = mybir.dt
    T, vt = meta["T"], meta["vt"]
    TT = meta["TT"]
    S = TT * 128
    Tmax = int(max(T))

    def din(name, shape, dtype):
        return nc.dram_tensor(name, shape, dtype, kind="ExternalInput").ap()

    xsT = din("xsT", [128, S], dt.float8e4)
    eaT = din("eaT", [65, S], dt.float8e4)
    ohd = din("ohd", [128, TT, 128], dt.float8e4)
    ohtd = din("ohtd", [128, S], dt.float8e4)
    wea = din("wea", [128, H], dt.bfloat16)
    web = din("web", [128, H], dt.bfloat16)
    wec = din("wec", [65, H], dt.bfloat16)
    we2 = din("we2", [128, H], dt.bfloat16)
    eb2c = din("eb2c", [128, 1], dt.float32)
    wna = din("wna", [128, H], dt.bfloat16)
    wnb = din("wnb", [128, H], dt.bfloat16)
    nbc = din("nbc", [2, H], dt.bfloat16)
    wn2 = din("wn2", [128, D], dt.bfloat16)
    nb2r = din("nb2r", [1, D], dt.bfloat16)
    degs = din("degs", [2, NPAD], dt.bfloat16)
    ngm = din("ngm", [128, 128], dt.float32)
    nbm = din("nbm", [128, 128], dt.float32)
    idn = din("idn", [128, 128], dt.bfloat16)
    onesw = din("onesw", [1, GRP * 128], dt.bfloat16)
    epsc = din("epsc", [128, 1], dt.float32)
    xtl = din("xtl", [128, NPAD], dt.bfloat16)
    out = nc.dram_tensor("out", [NPAD, D], dt.float32, kind="ExternalOutput").ap()

    with tile.TileContext(nc) as tc:
        with (
            tc.tile_pool(name="singles", bufs=1) as singles,
            tc.tile_pool(name="aggp", bufs=1) as aggp,
            tc.tile_pool(name="blockin", bufs=2) as blockin,
            tc.tile_pool(name="work", bufs=3) as work,
            tc.tile_pool(name="nwork", bufs=3) as nwork,
            tc.tile_pool(name="ph1", bufs=2, space="PSUM") as ph1,
            tc.tile_pool(name="ph2", bufs=2, space="PSUM") as ph2,
            tc.tile_pool(name="ptr", bufs=2, space="PSUM") as ptr,
            tc.tile_pool(name="pagg", bufs=2, space="PSUM") as pagg,
        ):
            def load(ap_, shape, dtype, tag):
                t = singles.tile(shape, dtype, tag=tag)
                nc.sync.dma_start(out=t[:], in_=ap_)
                return t

            def load_block(j):
                Tj = T[j]
                vt0 = vt[j]
                xsb = blockin.tile([128, Tmax * 128], dt.float8e4, tag="xsb")
                nc.sync.dma_start(out=xsb[:, :Tj * 128],
                                  in_=xsT[:, vt0 * 128:(vt0 + Tj) * 128])
                eab = blockin.tile([65, Tmax * 128], dt.float8e4, tag="eab")
                nc.sync.dma_start(out=eab[:, :Tj * 128],
                                  in_=eaT[:, vt0 * 128:(vt0 + Tj) * 128])
                ohb = blockin.tile([128, Tmax, 128], dt.float8e4, tag="ohb")
                nc.sync.dma_start(out=ohb[:, :Tj, :],
                                  in_=ohd[:, vt0:vt0 + Tj, :])
                ohtb = blockin.tile([128, Tmax * 128], dt.float8e4, tag="ohtb")
                nc.sync.dma_start(out=ohtb[:, :Tj * 128],
                                  in_=ohtd[:, vt0 * 128:(vt0 + Tj) * 128])
                return xsb, eab, ohb, ohtb

            # edge-critical weights first, then the first blocks' streams,
            # THEN the bulky node-phase singles -- so compute starts early
            s_wea = load(wea, [128, H], dt.bfloat16, "wea")
            s_web = load(web, [128, H], dt.bfloat16, "web")
            s_wec = load(wec, [65, H], dt.bfloat16, "wec")
            s_we2 = load(we2, [128, H], dt.bfloat16, "we2")
            s_eb2c = load(eb2c, [128, 1], dt.float32, "eb2c")
            s_idn = load(idn, [128, 128], dt.bfloat16, "idn")
            s_eps = load(epsc, [128, 1], dt.float32, "epsc")
            s_xtl = load(xtl, [128, NPAD], dt.bfloat16, "xtl")
            prefetched = {j: load_block(j) for j in (0, 1)}
            s_wna = load(wna, [128, H], dt.bfloat16, "wna")
            s_wnb = load(wnb, [128, H], dt.bfloat16, "wnb")
            s_nbc = load(nbc, [2, H], dt.bfloat16, "nbc")
            s_wn2 = load(wn2, [128, D], dt.bfloat16, "wn2")
            s_nb2r = load(nb2r, [1, D], dt.bfloat16, "nb2r")
            s_degs = load(degs, [2, NPAD], dt.bfloat16, "degs")
            s_ngm = load(ngm, [128, 128], dt.float32, "ngm")
            s_nbm = load(nbm, [128, 128], dt.float32, "nbm")
            s_onesw = load(onesw, [1, GRP * 128], dt.bfloat16, "onesw")

            s_agg = aggp.tile([128, NPAD], dt.bfloat16)   # agg^T
            s_zb = aggp.tile([128, NPAD], dt.bfloat16, tag="zb")  # (x@web)^T

            # ---- zb precompute: zb[j] = x_block @ web, 4 blocks per PSUM tile
            for j0 in range(0, NB, 4):
                nj = min(4, NB - j0)
                p4 = ph2.tile([128, GRP * 128], dt.float32, tag="h2")
                for b in range(nj):
                    nc.tensor.matmul(
                        out=p4[:, b * 128:(b + 1) * 128],
                        lhsT=s_xtl[:, (j0 + b) * 128:(j0 + b + 1) * 128],
                        rhs=s_web[:], start=True, stop=True)
                nc.scalar.copy(out=s_zb[:, j0 * 128:(j0 + nj) * 128],
                               in_=p4[:, :nj * 128])

            # ================= edge phase =================
            def edge_block(j):
                Tj = T[j]
                xsb, eab, ohb, ohtb = prefetched.pop(j, None) or load_block(j)

                p_agg = pagg.tile([128, 128], dt.float32, tag="agg")
                npair = sum(
                    (min(GRP, Tj - q0) + 1) // 2 for q0 in range(0, Tj, GRP))
                pc = 0
                for q0 in range(0, Tj, GRP):
                    nq = min(GRP, Tj - q0)
                    F = nq * 128

                    # h1^T = wea^T xs^T + zb^T onehot^T + wec^T ea
                    p_h1 = ph1.tile([128, GRP * 128], dt.float32, tag="h1")
                    nc.tensor.matmul(
                        out=p_h1[:, :F], lhsT=s_wea[:],
                        rhs=xsb[:, q0 * 128:q0 * 128 + F],
                        start=True, stop=False)
                    nc.tensor.matmul(
                        out=p_h1[:, :F],
                        lhsT=s_zb[:, j * 128:(j + 1) * 128],
                        rhs=ohtb[:, q0 * 128:q0 * 128 + F],
                        start=False, stop=False)
                    nc.tensor.matmul(
                        out=p_h1[:, :F], lhsT=s_wec[:],
                        rhs=eab[:, q0 * 128:q0 * 128 + F],
                        start=False, stop=True)
                    h1r = work.tile([128, GRP * 128], dt.bfloat16, tag="h1r")
                    nc.scalar.activation(
                        out=h1r[:, :F], in_=p_h1[:, :F],
                        func=mybir.ActivationFunctionType.Relu)

                    # h2^T = we2^T relu(h1^T); relu(+eb2) on copy-out
                    p_h2 = ph2.tile([128, GRP * 128], dt.float32, tag="h2")
                    nc.tensor.matmul(out=p_h2[:, :F], lhsT=s_we2[:],
                                     rhs=h1r[:, :F], start=True, stop=True)
                    rT = work.tile([128, GRP * 128], dt.bfloat16, tag="rT")
                    nc.scalar.activation(
                        out=rT[:, :F], in_=p_h2[:, :F],
                        func=mybir.ActivationFunctionType.Relu,
                        bias=s_eb2c[:, 0:1])

                    # per-tile transpose to [edge, feat]; LN over features
                    tr = ptr.tile([128, 8, 128], dt.bfloat16, tag="tr")
                    for q in range(nq):
                        nc.tensor.transpose(
                            out=tr[:, 4 + q, :],
                            in_=rT[:, q * 128:(q + 1) * 128],
                            identity=s_idn[:])
                    st = work.tile([128, GRP, 6], dt.float32, tag="st")
                    mv = work.tile([128, GRP, 2], dt.float32, tag="mv")
                    sd = work.tile([128, GRP], dt.float32, tag="sd")
                    inv = work.tile([128, GRP], dt.float32, tag="inv")
                    nmi = work.tile([128, GRP], dt.float32, tag="nmi")
                    msg = work.tile([128, GRP * 128], dt.float8e4, tag="msg")
                    for q in range(nq):
                        nc.vector.bn_stats(out=st[:, q, :], in_=tr[:, 4 + q, :])
                        nc.vector.bn_aggr(out=mv[:, q, :], in_=st[:, q, :])
                    nc.scalar.activation(
                        out=sd[:, :nq], in_=mv[:, :nq, 1],
                        func=mybir.ActivationFunctionType.Sqrt,
                        bias=s_eps[:, 0:1])
                    nc.vector.reciprocal(out=inv[:, :nq], in_=sd[:, :nq])
                    # -mu * inv, for the scalar-engine normalize variant
                    nc.vector.scalar_tensor_tensor(
                        out=nmi[:, :nq], in0=mv[:, :nq, 0], scalar=-1.0,
                        in1=inv[:, :nq],
                        op0=mybir.AluOpType.mult, op1=mybir.AluOpType.mult)
                    for q in range(nq):
                        if q % 2 == 0:
                            nc.vector.tensor_scalar(
                                out=msg[:, q * 128:(q + 1) * 128],
                                in0=tr[:, 4 + q, :],
                                scalar1=mv[:, q, 0:1], scalar2=inv[:, q:q + 1],
                                op0=mybir.AluOpType.subtract,
                                op1=mybir.AluOpType.mult)
                        else:
                            nc.scalar.activation(
                                out=msg[:, q * 128:(q + 1) * 128],
                                in_=tr[:, 4 + q, :],
                                func=mybir.ActivationFunctionType.Identity,
                                scale=inv[:, q:q + 1], bias=nmi[:, q:q + 1])
                    # scatter: fp8 DoubleRow over tile pairs
                    q = 0
                    while q < nq:
                        if q + 1 < nq:
                            nc.tensor.matmul(
                                out=p_agg[:],
                                lhsT=msg[:, q * 128:(q + 2) * 128].rearrange(
                                    "p (t f) -> p t f", t=2),
                                rhs=ohb[:, q0 + q:q0 + q + 2, :],
                                perf_mode=mybir.MatmulPerfMode.DoubleRow,
                                start=(pc == 0), stop=(pc == npair - 1))
                            q += 2
                        else:
                            nc.tensor.matmul(
                                out=p_agg[:],
                                lhsT=msg[:, q * 128:(q + 1) * 128],
                                rhs=ohb[:, q0 + q, :],
                                start=(pc == 0), stop=(pc == npair - 1))
                            q += 1
                        pc += 1
                nc.scalar.copy(out=s_agg[:, j * 128:(j + 1) * 128], in_=p_agg[:])

            # ================= node phase =================
            def node_group(g):
                nj = min(GRP, NB - g)
                F = nj * 128
                c0 = g * 128
                p_hn = ph1.tile([128, GRP * 128], dt.float32, tag="h1")
                nc.tensor.matmul(out=p_hn[:, :F], lhsT=s_wna[:],
                                 rhs=s_xtl[:, c0:c0 + F], start=True, stop=False)
                nc.tensor.matmul(out=p_hn[:, :F], lhsT=s_wnb[:],
                                 rhs=s_agg[:, c0:c0 + F], start=False, stop=False)
                nc.tensor.matmul(out=p_hn[:, :F], lhsT=s_nbc[:],
                                 rhs=s_degs[:, c0:c0 + F], start=False, stop=True)
                hnr = nwork.tile([128, GRP * 128], dt.bfloat16, tag="hnr")
                nc.scalar.activation(out=hnr[:, :F], in_=p_hn[:, :F],
                                     func=mybir.ActivationFunctionType.Relu)
                p_up = ph2.tile([128, GRP * 128], dt.float32, tag="h2")
                nc.tensor.matmul(out=p_up[:, :F], lhsT=s_wn2[:],
                                 rhs=hnr[:, :F], start=True, stop=False)
                nc.tensor.matmul(out=p_up[:, :F], lhsT=s_nb2r[:],
                                 rhs=s_onesw[:, :F], start=False, stop=False)
                # fold the residual: p_up += x^T (idn @ xtl), so the
                # transposed tiles below are v = (x + upd) directly
                nc.tensor.matmul(out=p_up[:, :F], lhsT=s_idn[:],
                                 rhs=s_xtl[:, c0:c0 + F], start=False, stop=True)
                upT = nwork.tile([128, GRP * 128], dt.bfloat16, tag="upT")
                nc.scalar.copy(out=upT[:, :F], in_=p_up[:, :F])

                st = nwork.tile([128, GRP, 6], dt.float32, tag="nst")
                mv = nwork.tile([128, GRP, 2], dt.float32, tag="nmv")
                sd = nwork.tile([128, GRP], dt.float32, tag="nsd")
                inv = nwork.tile([128, GRP], dt.float32, tag="ninv")
                nrm = nwork.tile([128, GRP, 128], dt.bfloat16, tag="nrm")
                of = nwork.tile([128, GRP, 128], dt.float32, tag="of")
                trn = ptr.tile([128, 8, 128], dt.bfloat16, tag="tr")
                for q in range(nj):
                    nc.tensor.transpose(out=trn[:, q, :],
                                        in_=upT[:, q * 128:(q + 1) * 128],
                                        identity=s_idn[:])
                for q in range(nj):
                    nc.vector.bn_stats(out=st[:, q, :], in_=trn[:, q, :])
                    nc.vector.bn_aggr(out=mv[:, q, :], in_=st[:, q, :])
                nc.scalar.activation(out=sd[:, :nj], in_=mv[:, :nj, 1],
                                     func=mybir.ActivationFunctionType.Sqrt,
                                     bias=s_eps[:, 0:1])
                nc.vector.reciprocal(out=inv[:, :nj], in_=sd[:, :nj])
                for q in range(nj):
                    nc.vector.tensor_scalar(
                        out=nrm[:, q, :], in0=trn[:, q, :],
                        scalar1=mv[:, q, 0:1], scalar2=inv[:, q:q + 1],
                        op0=mybir.AluOpType.subtract, op1=mybir.AluOpType.mult)
                nc.vector.tensor_tensor(
                    out=nrm[:, :nj, :], in0=nrm[:, :nj, :],
                    in1=s_ngm[:, None, :].broadcast_to([128, nj, 128]),
                    op=mybir.AluOpType.mult)
                nc.vector.tensor_tensor(
                    out=of[:, :nj, :], in0=nrm[:, :nj, :],
                    in1=s_nbm[:, None, :].broadcast_to([128, nj, 128]),
                    op=mybir.AluOpType.add)
                nc.sync.dma_start(
                    out=out[c0:c0 + F, :].rearrange("(q p) f -> p q f", p=128),
                    in_=of[:, :nj, :])

            # lagged interleave: node group g issues well after its 4 agg
            # blocks completed, filling engine gaps without stalling the
            # in-order tensor queue on fresh s_agg copies
            LAG = 8
            done = 0
            for j in range(NB):
                edge_block(j)
                g = j - LAG + 1
                if g >= 0 and g % GRP == 0 and g + GRP <= j + 1:
                    node_group(g)
                    done = g + GRP
            for g in range(done, NB, GRP):
                node_group(g)
    return nc


def kernel(x, edge_index, edge_attr, eW1, eb1, eW2, eb2,
           nW1, nb1, nW2, nb2, eg, ebeta, ng, nbeta, _trace=False, _tmpdir=None):
    in_maps, meta = _host_prep(x, edge_index, edge_attr, eW1, eb1, eW2, eb2,
                               nW1, nb1, nW2, nb2, eg, ebeta, ng, nbeta)
    nc = bacc.Bacc("TRN2", target_bir_lowering=False, debug=False)
    build_program(nc, meta)
    nc.compile()
    res = run_bass_kernel_spmd(nc, in_maps, list(range(NC)), tmpdir=_tmpdir,
                               trace=_trace, trace_cores=[0] if _trace else None)
    full = np.empty((N, D), np.float32)
    for k in range(NC):
        o = res.results[k]["out"]                    # [NPAD, D], permuted rows
        porder = meta["porders"][k]
        valid = porder < NLOC
        full[k * NLOC + porder[valid]] = o[valid]
    kernel._last_results = res
    return full


# revision 29
# speedup vs baseline: 1.0380x; 1.0380x over previous
"""GNS layer (edge MLP -> segment_sum -> node MLP + layernorms) on 8 trn2 cores.

Sharding: edges partitioned by *receiver* node range; core k owns nodes
[6250k, 6250(k+1)) and the edges whose receiver lands there. Local nodes are
permuted into 49 blocks of 128 via a capacity-constrained degree-balanced
packing so nearly every block needs exactly ceil(E_core/(49*128)) edge tiles;
the segment-sum is core-local (PSUM one-hot scatter) and the host un-permutes
the output rows.

Edge pipeline runs "transposed" ([feat, edge] layouts) so the big MLP matmuls
keep weights stationary and stream 512-edge tiles. Sender features are
pre-gathered on the host into a [128, S] fp8 stream (x[senders].T in edge
order) -- no on-device gather. Receiver contribution uses the zb trick:
zb = x_block @ eW1[D:2D] computed on-device per block, scattered to edges via
a one-hot matmul. One-hots are host-built, streamed in fp8 (exact for 0/1).
All matmuls run plain bf16-rate (fp8 DoubleRow tripped the chip power
throttle and was a net loss).

Host prep (cheap, numpy): sort edges by receiver, balanced node permutation,
pad each (core, block) to the shared tile profile so all 8 cores run one
SPMD program; pre-gather/transpose sender features and edge_attr; fold the
edge-layernorm affine (eg, ebeta) exactly into the node-MLP weights.
"""

import sys

sys.path.insert(0, "/opt/trn_rl_repo")

import numpy as np
import ml_dtypes

import concourse.bacc as bacc
import concourse.bass as bass
import concourse.tile as tile
from concourse import mybir
from concourse.bass_utils import run_bass_kernel_spmd

BF16 = ml_dtypes.bfloat16
FP8 = ml_dtypes.float8_e4m3

N, E, D, A, H = 50000, 500000, 128, 64, 128
NC = 8
NLOC = N // NC            # 6250 nodes per core
NB = 49                   # blocks of 128 local nodes (49*128 = 6272 >= 6250)
NPAD = NB * 128           # 6272
EPS = 1e-5
GRP = 4                   # edge tiles batched per pointwise group


def _pack_blocks(deg, T):
    """Assign NPAD nodes (with edge counts deg) to NB blocks of exactly 128
    nodes with per-block edge capacity T[j]*128. Returns [NB, 128] local node
    ids, or None if the capacity is infeasible for this profile."""
    C = T * 128
    order = np.argsort(-deg, kind="stable")
    sums = np.zeros(NB, np.int64)
    cnts = np.zeros(NB, np.int64)
    members = [[] for _ in range(NB)]
    for n in order:
        cand = np.flatnonzero(cnts < 128)
        j = cand[np.argmax(C[cand] - sums[cand])]
        members[j].append(n)
        sums[j] += deg[n]
        cnts[j] += 1
    if (sums > C).any():
        return None
    return np.array(members, dtype=np.int64)


def _host_prep(x, edge_index, edge_attr, eW1, eb1, eW2, eb2,
               nW1, nb1, nW2, nb2, eg, ebeta, ng, nbeta):
    x = np.asarray(x, np.float32)
    s = np.asarray(edge_index[0], np.int64).astype(np.int32)
    r = np.asarray(edge_index[1], np.int64).astype(np.int32)
    ea = np.asarray(edge_attr, np.float32)

    perm = np.argsort(r, kind="stable")
    rs, ss = r[perm], s[perm]
    eas = ea[perm]

    xbf = x.astype(BF16)

    # ---- per-core degree info
    degs_k, ptrs_k = [], []
    for k in range(NC):
        base = k * NLOC
        ptr = np.searchsorted(rs, np.arange(base, base + NLOC + 1))
        d = ptr[1:] - ptr[:-1]
        degs_k.append(np.concatenate([d, np.zeros(NPAD - NLOC, np.int64)]))
        ptrs_k.append(ptr)

    # shared tile profile: n11 blocks of 11 tiles, rest 10 (auto-bumped)
    tiles_needed = max(max(NB, int(np.ceil(dd.sum() / 128))) for dd in degs_k)
    base_t = max(1, tiles_needed // NB)
    n_hi = tiles_needed - base_t * NB
    for bump in range(NB + 1):
        nh = min(NB, n_hi + bump)
        T = np.array([base_t + 1] * nh + [base_t] * (NB - nh), np.int64)
        blocks = [_pack_blocks(degs_k[k], T) for k in range(NC)]
        if all(b is not None for b in blocks):
            break
    assert all(b is not None for b in blocks)

    vt = np.concatenate([[0], np.cumsum(T)])
    TT = int(vt[-1])
    S = TT * 128

    eW1 = np.asarray(eW1, np.float32)
    nW1 = np.asarray(nW1, np.float32)
    wec = np.concatenate([eW1[2 * D:], np.asarray(eb1, np.float32)[None, :]], 0)
    nW1b_eff = np.asarray(eg, np.float32)[:, None] * nW1[D:]
    c_row = np.asarray(ebeta, np.float32) @ nW1[D:]
    nbc = np.stack([np.asarray(nb1, np.float32), c_row], 0)  # [2, H]

    idn = np.eye(128, dtype=np.float32)

    common = dict(
        wea=eW1[:D].astype(BF16), web=eW1[D:2 * D].astype(BF16),
        wec=wec.astype(BF16), we2=np.asarray(eW2, np.float32).astype(BF16),
        eb2c=np.asarray(eb2, np.float32)[:, None].copy(),   # [128, 1] f32
        wna=nW1[:D].astype(BF16), wnb=nW1b_eff.astype(BF16),
        nbc=nbc.astype(BF16),
        wn2=np.asarray(nW2, np.float32).astype(BF16),
        nb2r=np.asarray(nb2, np.float32)[None, :].astype(BF16),  # [1, D]
        ngm=np.broadcast_to(np.asarray(ng, np.float32), (128, 128)).copy(),
        nbm=np.broadcast_to(np.asarray(nbeta, np.float32), (128, 128)).copy(),
        idn=idn.astype(BF16),
        onesw=np.ones((1, GRP * 128), BF16),
        epsc=np.full((128, 1), EPS, np.float32),
    )
    in_maps = []
    xposed = np.ascontiguousarray(x.T)               # [128, N] f32
    porders = []
    for k in range(NC):
        base = k * NLOC
        ptr = ptrs_k[k]
        d = ptr[1:] - ptr[:-1]
        bl = blocks[k]                               # [NB, 128] local ids

        porder = bl.reshape(-1)                      # slot -> local node id
        porders.append(porder)
        gids = np.minimum(porder + base, N - 1)
        deg_full = degs_k[k]
        degs = np.zeros((2, NPAD), np.float32)
        degs[0, :] = 1.0
        degs[1, :] = deg_full[porder]
        xtl = xposed[:, gids].astype(BF16)           # [128, NPAD]

        # edge slots, per block in slot order
        tot_e = int(d.sum())
        cols = np.empty(tot_e, np.int64)
        rloc = np.empty(tot_e, np.int64)
        eidx = np.empty(tot_e, np.int64)
        pos = 0
        for j in range(NB):
            nl = bl[j]
            real = nl < NLOC
            nlr = nl[real]
            lens = d[nlr]
            tot = int(lens.sum())
            if tot:
                starts = ptr[:-1][nlr]
                off = np.repeat(
                    starts - np.concatenate([[0], np.cumsum(lens)[:-1]]), lens)
                e = off + np.arange(tot)
                slots = np.repeat(np.flatnonzero(real), lens)
                cols[pos:pos + tot] = vt[j] * 128 + np.arange(tot)
                rloc[pos:pos + tot] = slots
                eidx[pos:pos + tot] = e
                pos += tot
        cols = cols[:pos]; rloc = rloc[:pos]; eidx = eidx[:pos]

        xsT = np.zeros((128, S), FP8)
        xsT[:, cols] = xbf[ss[eidx]].astype(FP8).T
        eaT = np.zeros((65, S), FP8)
        eaT[64, :] = 1.0
        eaT[:64, cols] = eas[eidx].astype(FP8).T
        ohd = np.zeros((128, TT, 128), FP8)
        ohd[cols % 128, cols // 128, rloc] = 1.0
        ohtd = np.zeros((128, S), FP8)
        ohtd[rloc, cols] = 1.0

        m = dict(common)
        m["xtl"] = xtl
        m["xsT"] = xsT
        m["eaT"] = eaT
        m["ohd"] = ohd
        m["ohtd"] = ohtd
        m["degs"] = degs.astype(BF16)
        in_maps.append(m)
    meta = dict(T=[int(v) for v in T], vt=[int(v) for v in vt], TT=TT,
                porders=porders)
    return in_maps, meta


def build_program(nc, meta):
    dt = mybir.dt
    T, vt = meta["T"], meta["vt"]
    TT = meta["TT"]
    S = TT * 128
    Tmax = int(max(T))

    def din(name, shape, dtype):
        return nc.dram_tensor(name, shape, dtype, kind="ExternalInput").ap()

    xsT = din("xsT", [128, S], dt.float8e4)
    eaT = din("eaT", [65, S], dt.float8e4)
    ohd = din("ohd", [128, TT, 128], dt.float8e4)
    ohtd = din("ohtd", [128, S], dt.float8e4)
    wea = din("wea", [128, H], dt.bfloat16)
    web = din("web", [128, H], dt.bfloat16)
    wec = din("wec", [65, H], dt.bfloat16)
    we2 = din("we2", [128, H], dt.bfloat16)
    eb2c = din("eb2c", [128, 1], dt.float32)
    wna = din("wna", [128, H], dt.bfloat16)
    wnb = din("wnb", [128, H], dt.bfloat16)
    nbc = din("nbc", [2, H], dt.bfloat16)
    wn2 = din("wn2", [128, D], dt.bfloat16)
    nb2r = din("nb2r", [1, D], dt.bfloat16)
    degs = din("degs", [2, NPAD], dt.bfloat16)
    ngm = din("ngm", [128, 128], dt.float32)
    nbm = din("nbm", [128, 128], dt.float32)
    idn = din("idn", [128, 128], dt.bfloat16)
    onesw = din("onesw", [1, GRP * 128], dt.bfloat16)
    epsc = din("epsc", [128, 1], dt.float32)
    xtl = din("xtl", [128, NPAD], dt.bfloat16)
    out = nc.dram_tensor("out", [NPAD, D], dt.float32, kind="ExternalOutput").ap()

    with tile.TileContext(nc) as tc:
        with (
            tc.tile_pool(name="singles", bufs=1) as singles,
            tc.tile_pool(name="aggp", bufs=1) as aggp,
            tc.tile_pool(name="blockin", bufs=2) as blockin,
            tc.tile_pool(name="work", bufs=3) as work,
            tc.tile_pool(name="nwork", bufs=3) as nwork,
            tc.tile_pool(name="ph1", bufs=2, space="PSUM") as ph1,
            tc.tile_pool(name="ph2", bufs=2, space="PSUM") as ph2,
            tc.tile_pool(name="ptr", bufs=2, space="PSUM") as ptr,
            tc.tile_pool(name="pagg", bufs=2, space="PSUM") as pagg,
        ):
            def load(ap_, shape, dtype, tag):
                t = singles.tile(shape, dtype, tag=tag)
                nc.sync.dma_start(out=t[:], in_=ap_)
                return t

            def load_block(j):
                Tj = T[j]
                vt0 = vt[j]
                xsb = blockin.tile([128, Tmax * 128], dt.float8e4, tag="xsb")
                nc.sync.dma_start(out=xsb[:, :Tj * 128],
                                  in_=xsT[:, vt0 * 128:(vt0 + Tj) * 128])
                eab = blockin.tile([65, Tmax * 128], dt.float8e4, tag="eab")
                nc.sync.dma_start(out=eab[:, :Tj * 128],
                                  in_=eaT[:, vt0 * 128:(vt0 + Tj) * 128])
                ohb = blockin.tile([128, Tmax, 128], dt.float8e4, tag="ohb")
                nc.sync.dma_start(out=ohb[:, :Tj, :],
                                  in_=ohd[:, vt0:vt0 + Tj, :])
                ohtb = blockin.tile([128, Tmax * 128], dt.float8e4, tag="ohtb")
                nc.sync.dma_start(out=ohtb[:, :Tj * 128],
                                  in_=ohtd[:, vt0 * 128:(vt0 + Tj) * 128])
                return xsb, eab, ohb, ohtb

            # edge-critical weights first, then the first blocks' streams,
            # THEN the bulky node-phase singles -- so compute starts early
            s_wea = load(wea, [128, H], dt.bfloat16, "wea")
            s_web = load(web, [128, H], dt.bfloat16, "web")
            s_wec = load(wec, [65, H], dt.bfloat16, "wec")
            s_we2 = load(we2, [128, H], dt.bfloat16, "we2")
            s_eb2c = load(eb2c, [128, 1], dt.float32, "eb2c")
            s_idn = load(idn, [128, 128], dt.bfloat16, "idn")
            s_eps = load(epsc, [128, 1], dt.float32, "epsc")
            s_xtl = load(xtl, [128, NPAD], dt.bfloat16, "xtl")
            prefetched = {j: load_block(j) for j in (0, 1)}
            s_wna = load(wna, [128, H], dt.bfloat16, "wna")
            s_wnb = load(wnb, [128, H], dt.bfloat16, "wnb")
            s_nbc = load(nbc, [2, H], dt.bfloat16, "nbc")
            s_wn2 = load(wn2, [128, D], dt.bfloat16, "wn2")
            s_nb2r = load(nb2r, [1, D], dt.bfloat16, "nb2r")
            s_degs = load(degs, [2, NPAD], dt.bfloat16, "degs")
            s_ngm = load(ngm, [128, 128], dt.float32, "ngm")
            s_nbm = load(nbm, [128, 128], dt.float32, "nbm")
            s_onesw = load(onesw, [1, GRP * 128], dt.bfloat16, "onesw")

            s_agg = aggp.tile([128, NPAD], dt.bfloat16)   # agg^T
            s_zb = aggp.tile([128, NPAD], dt.bfloat16, tag="zb")  # (x@web)^T

            # ---- zb precompute: zb[j] = x_block @ web, 4 blocks per PSUM tile
            for j0 in range(0, NB, 4):
                nj = min(4, NB - j0)
                p4 = ph2.tile([128, GRP * 128], dt.float32, tag="h2")
                for b in range(nj):
                    nc.tensor.matmul(
                        out=p4[:, b * 128:(b + 1) * 128],
                        lhsT=s_xtl[:, (j0 + b) * 128:(j0 + b + 1) * 128],
                        rhs=s_web[:], start=True, stop=True)
                nc.scalar.copy(out=s_zb[:, j0 * 128:(j0 + nj) * 128],
                               in_=p4[:, :nj * 128])

            # ================= edge phase =================
            def edge_block(j):
                Tj = T[j]
                xsb, eab, ohb, ohtb = prefetched.pop(j, None) or load_block(j)

                p_agg = pagg.tile([128, 128], dt.float32, tag="agg")
                tc_i = 0
                for q0 in range(0, Tj, GRP):
                    nq = min(GRP, Tj - q0)
                    F = nq * 128

                    # h1^T = wea^T xs^T + zb^T onehot^T + wec^T ea
                    p_h1 = ph1.tile([128, GRP * 128], dt.float32, tag="h1")
                    nc.tensor.matmul(
                        out=p_h1[:, :F], lhsT=s_wea[:],
                        rhs=xsb[:, q0 * 128:q0 * 128 + F],
                        start=True, stop=False)
                    nc.tensor.matmul(
                        out=p_h1[:, :F],
                        lhsT=s_zb[:, j * 128:(j + 1) * 128],
                        rhs=ohtb[:, q0 * 128:q0 * 128 + F],
                        start=False, stop=False)
                    nc.tensor.matmul(
                        out=p_h1[:, :F], lhsT=s_wec[:],
                        rhs=eab[:, q0 * 128:q0 * 128 + F],
                        start=False, stop=True)
                    h1r = work.tile([128, GRP * 128], dt.bfloat16, tag="h1r")
                    nc.scalar.activation(
                        out=h1r[:, :F], in_=p_h1[:, :F],
                        func=mybir.ActivationFunctionType.Relu)

                    # h2^T = we2^T relu(h1^T); relu(+eb2) on copy-out
                    p_h2 = ph2.tile([128, GRP * 128], dt.float32, tag="h2")
                    nc.tensor.matmul(out=p_h2[:, :F], lhsT=s_we2[:],
                                     rhs=h1r[:, :F], start=True, stop=True)
                    rT = work.tile([128, GRP * 128], dt.bfloat16, tag="rT")
                    nc.scalar.activation(
                        out=rT[:, :F], in_=p_h2[:, :F],
                        func=mybir.ActivationFunctionType.Relu,
                        bias=s_eb2c[:, 0:1])

                    # per-tile transpose to [edge, feat]; LN over features
                    tr = ptr.tile([128, 8, 128], dt.bfloat16, tag="tr")
                    for q in range(nq):
                        nc.tensor.transpose(
                            out=tr[:, 4 + q, :],
                            in_=rT[:, q * 128:(q + 1) * 128],
                            identity=s_idn[:])
                    st = work.tile([128, GRP, 6], dt.float32, tag="st")
                    mv = work.tile([128, GRP, 2], dt.float32, tag="mv")
                    sd = work.tile([128, GRP], dt.float32, tag="sd")
                    inv = work.tile([128, GRP], dt.float32, tag="inv")
                    nmi = work.tile([128, GRP], dt.float32, tag="nmi")
                    msg = work.tile([128, GRP * 128], dt.bfloat16, tag="msg")
                    for q in range(nq):
                        nc.vector.bn_stats(out=st[:, q, :], in_=tr[:, 4 + q, :])
                        nc.vector.bn_aggr(out=mv[:, q, :], in_=st[:, q, :])
                    nc.scalar.activation(
                        out=sd[:, :nq], in_=mv[:, :nq, 1],
                        func=mybir.ActivationFunctionType.Sqrt,
                        bias=s_eps[:, 0:1])
                    nc.vector.reciprocal(out=inv[:, :nq], in_=sd[:, :nq])
                    # -mu * inv, for the scalar-engine normalize variant
                    nc.vector.scalar_tensor_tensor(
                        out=nmi[:, :nq], in0=mv[:, :nq, 0], scalar=-1.0,
                        in1=inv[:, :nq],
                        op0=mybir.AluOpType.mult, op1=mybir.AluOpType.mult)
                    for q in range(nq):
                        if q % 2 == 0:
                            nc.vector.tensor_scalar(
                                out=msg[:, q * 128:(q + 1) * 128],
                                in0=tr[:, 4 + q, :],
                                scalar1=mv[:, q, 0:1], scalar2=inv[:, q:q + 1],
                                op0=mybir.AluOpType.subtract,
                                op1=mybir.AluOpType.mult)
                        else:
                            nc.scalar.activation(
                                out=msg[:, q * 128:(q + 1) * 128],
                                in_=tr[:, 4 + q, :],
                                func=mybir.ActivationFunctionType.Identity,
                                scale=inv[:, q:q + 1], bias=nmi[:, q:q + 1])
                        nc.tensor.matmul(
                            out=p_agg[:],
                            lhsT=msg[:, q * 128:(q + 1) * 128],
                            rhs=ohb[:, q0 + q, :],
                            start=(tc_i == 0), stop=(tc_i == Tj - 1))
                        tc_i += 1
                nc.scalar.copy(out=s_agg[:, j * 128:(j + 1) * 128], in_=p_agg[:])

            # ================= node phase =================
            def node_group(g):
                nj = min(GRP, NB - g)
                F = nj * 128
                c0 = g * 128
                p_hn = ph1.tile([128, GRP * 128], dt.float32, tag="h1")
                nc.tensor.matmul(out=p_hn[:, :F], lhsT=s_wna[:],
                                 rhs=s_xtl[:, c0:c0 + F], start=True, stop=False)
                nc.tensor.matmul(out=p_hn[:, :F], lhsT=s_wnb[:],
                                 rhs=s_agg[:, c0:c0 + F], start=False, stop=False)
                nc.tensor.matmul(out=p_hn[:, :F], lhsT=s_nbc[:],
                                 rhs=s_degs[:, c0:c0 + F], start=False, stop=True)
                hnr = nwork.tile([128, GRP * 128], dt.bfloat16, tag="hnr")
                nc.scalar.activation(out=hnr[:, :F], in_=p_hn[:, :F],
                                     func=mybir.ActivationFunctionType.Relu)
                p_up = ph2.tile([128, GRP * 128], dt.float32, tag="h2")
                nc.tensor.matmul(out=p_up[:, :F], lhsT=s_wn2[:],
                                 rhs=hnr[:, :F], start=True, stop=False)
                nc.tensor.matmul(out=p_up[:, :F], lhsT=s_nb2r[:],
                                 rhs=s_onesw[:, :F], start=False, stop=False)
                # fold the residual: p_up += x^T (idn @ xtl), so the
                # transposed tiles below are v = (x + upd) directly
                nc.tensor.matmul(out=p_up[:, :F], lhsT=s_idn[:],
                                 rhs=s_xtl[:, c0:c0 + F], start=False, stop=True)
                upT = nwork.tile([128, GRP * 128], dt.bfloat16, tag="upT")
                nc.scalar.copy(out=upT[:, :F], in_=p_up[:, :F])

                st = nwork.tile([128, GRP, 6], dt.float32, tag="nst")
                mv = nwork.tile([128, GRP, 2], dt.float32, tag="nmv")
                sd = nwork.tile([128, GRP], dt.float32, tag="nsd")
                inv = nwork.tile([128, GRP], dt.float32, tag="ninv")
                nrm = nwork.tile([128, GRP, 128], dt.bfloat16, tag="nrm")
                of = nwork.tile([128, GRP, 128], dt.float32, tag="of")
                trn = ptr.tile([128, 8, 128], dt.bfloat16, tag="tr")
                for q in range(nj):
                    nc.tensor.transpose(out=trn[:, q, :],
                                        in_=upT[:, q * 128:(q + 1) * 128],
                                        identity=s_idn[:])
                for q in range(nj):
                    nc.vector.bn_stats(out=st[:, q, :], in_=trn[:, q, :])
                    nc.vector.bn_aggr(out=mv[:, q, :], in_=st[:, q, :])
                nc.scalar.activation(out=sd[:, :nj], in_=mv[:, :nj, 1],
                                     func=mybir.ActivationFunctionType.Sqrt,
                                     bias=s_eps[:, 0:1])
                nc.vector.reciprocal(out=inv[:, :nj], in_=sd[:, :nj])
                for q in range(nj):
                    nc.vector.tensor_scalar(
                        out=nrm[:, q, :], in0=trn[:, q, :],
                        scalar1=mv[:, q, 0:1], scalar2=inv[:, q:q + 1],
                        op0=mybir.AluOpType.subtract, op1=mybir.AluOpType.mult)
                nc.vector.tensor_tensor(
                    out=nrm[:, :nj, :], in0=nrm[:, :nj, :],
                    in1=s_ngm[:, None, :].broadcast_to([128, nj, 128]),
                    op=mybir.AluOpType.mult)
                nc.vector.tensor_tensor(
                    out=of[:, :nj, :], in0=nrm[:, :nj, :],
                    in1=s_nbm[:, None, :].broadcast_to([128, nj, 128]),
                    op=mybir.AluOpType.add)
                nc.sync.dma_start(
                    out=out[c0:c0 + F, :].rearrange("(q p) f -> p q f", p=128),
                    in_=of[:, :nj, :])

            # lagged interleave: node group g issues well after its 4 agg
            # blocks completed, filling engine gaps without stalling the
            # in-order tensor queue on fresh s_agg copies
            LAG = 8
            done = 0
            for j in range(NB):
                edge_block(j)
                g = j - LAG + 1
                if g >= 0 and g % GRP == 0 and g + GRP <= j + 1:
                    node_group(g)
                    done = g + GRP
            for g in range(done, NB, GRP):
                node_group(g)
    return nc


def kernel(x, edge_index, edge_attr, eW1, eb1, eW2, eb2,
           nW1, nb1, nW2, nb2, eg, ebeta, ng, nbeta, _trace=False, _tmpdir=None):
    in_maps, meta = _host_prep(x, edge_index, edge_attr, eW1, eb1, eW2, eb2,
                               nW1, nb1, nW2, nb2, eg, ebeta, ng, nbeta)
    nc = bacc.Bacc("TRN2", target_bir_lowering=False, debug=False)
    build_program(nc, meta)
    nc.compile()
    res = run_bass_kernel_spmd(nc, in_maps, list(range(NC)), tmpdir=_tmpdir,
                               trace=_trace, trace_cores=[0] if _trace else None)
    full = np.empty((N, D), np.float32)
    for k in range(NC):
        o = res.results[k]["out"]                    # [NPAD, D], permuted rows
        porder = meta["porders"][k]
        valid = porder < NLOC
        full[k * NLOC + porder[valid]] = o[valid]
    kernel._last_results = res
    return full
